# revision 3
# baseline (speedup 1.0000x reference)
"""Trainium2 Bass kernel: memory-slot cross-attention (nn_LocalConstructorMulti).

Algebraic restructuring vs the reference:
    scores[b,h,n,s] = (Q[n,h,:] . K[b,s,h,:]) / 8
                    = hs[b,s,:] . qe[h*8+n,:]        with qe = fold(Wk, Wq, ms)
    out[b,n,h,:]    = Wv_h @ ctx[b,h*8+n,:]          with ctx = attn-weighted
                                                     sum of hidden states
    y[b,n,:]        = sum_h Wo_h (Wv_h ctx_hn)

So the device only computes, per (batch, seq-half) core:
    Phase 1: s[hn, s]  = qe8.T @ hsT8   (fp8, DoubleRow)   [64, 2048]
             p~[hn, s] = exp(s * EXPSCALE + maskbias)      (unnormalized)
    Phase 2: ctx[hn,:] = p~ @ hs        (bf16)             [64, 4096]
             den[hn]   = sum_s p~                          (ACT accum_out)
The tiny Wv/Wo projections and the 1/den normalization happen on host
(linear ops commute with the attention sum; den differs per (h,n) so the
normalization must precede the Wo mix, after summing the two seq-halves).

This removes the big K/V projections entirely: device work drops from
~34 GFLOP/core to ~4.3 GFLOP/core and the kernel becomes DMA-bound on
reading hs once in bf16 (16.8 MB) plus once transposed in fp8e4m3
(8.4 MB; fp8 is accurate enough for scores because the per-element
quantization noise averages over the 4096-long contraction while the
score scale is ~1/64).

Sharding: 8 cores = 4 batches x 2 sequence halves. Host sums the two
halves' unnormalized ctx/den, normalizes, and applies Wv/Wo.
"""

import sys

if "/opt/trn_rl_repo" not in sys.path:
    sys.path.insert(0, "/opt/trn_rl_repo")

import ml_dtypes
import numpy as np

import concourse.bass as bass  # noqa: F401  (AP helpers)
import concourse.mybir as mybir
import concourse.tile as tile
from concourse import bacc
from concourse.bass_utils import run_bass_kernel_spmd
from concourse.masks import make_identity

BF16 = mybir.dt.bfloat16
F32 = mybir.dt.float32
F8 = mybir.dt.float8e4
npbf16 = ml_dtypes.bfloat16
npf8 = ml_dtypes.float8_e4m3

B, S, HID = 4, 4096, 4096
SLOTS, HEADS, BD = 8, 8, 512
HD = BD // HEADS  # 64
N_CORES = 8
HALVES = 2
SH = S // HALVES  # 2048 sequence positions per core
HN = HEADS * SLOTS  # 64 (head, slot) pairs
NK = HID // 128  # 32 contraction k-tiles
NST = SH // 128  # 16 seq row-tiles
CHW = 512  # score chunk width (one PSUM bank of fp32)
NCH = SH // CHW  # 4 score chunks
NCG = HID // 512  # 8 ctx col groups (PSUM banks)
QSCALE = 512.0  # qe pre-scale so fp8 values sit in the normal range
SCALE = 1.0 / float(np.sqrt(HD))
EXPSCALE = 1.0 / (QSCALE / SCALE)  # undo QSCALE, apply 1/sqrt(HD)
MASK_NEG = -1.0e9  # after *EXPSCALE -> -244k -> exp -> 0

# test.py can flip this to capture an NTFF profile; harness never touches it.
TRACE = False
TRACE_CORES = None
LAST_RESULT = None

_cache = {}


def _build_module():
    """Emit + compile the single-core Bass module (same NEFF on all cores)."""
    nc = bacc.Bacc("TRN2", target_bir_lowering=False, debug=False, num_devices=N_CORES)

    hsT8 = nc.dram_tensor("hsT8", [HID, SH], F8, kind="ExternalInput").ap()
    hs16 = nc.dram_tensor("hs16", [SH, HID], BF16, kind="ExternalInput").ap()
    qe8 = nc.dram_tensor("qe8", [128, NK * HN], F8, kind="ExternalInput").ap()
    mbT = nc.dram_tensor("mbT", [1, SH], BF16, kind="ExternalInput").ap()
    ctx_out = nc.dram_tensor("ctx_out", [HN, HID], F32, kind="ExternalOutput").ap()
    den_out = nc.dram_tensor("den_out", [HN, 1], F32, kind="ExternalOutput").ap()

    with tile.TileContext(nc) as tc:
        with (
            tc.tile_pool(name="consts", bufs=1) as consts,
            tc.tile_pool(name="hstp", bufs=2) as hstp,
            tc.tile_pool(name="hsp", bufs=6) as hsp,
        ):
            # ---- small resident tensors ---------------------------------
            qe_sb = consts.tile([128, NK, HN], F8)
            nc.sync.dma_start(
                out=qe_sb, in_=qe8.rearrange("p (a b) -> p a b", a=NK)
            )
            mb_sb = consts.tile([1, SH], BF16)
            nc.sync.dma_start(out=mb_sb, in_=mbT)
            ones_sb = consts.tile([1, HN], BF16)
            nc.vector.memset(ones_sb, 1.0)
            ident = consts.tile([HN, HN], BF16)
            make_identity(nc, ident)
            pT_sb = consts.tile([128, NST, HN], BF16)
            den4 = consts.tile([HN, NCH], F32)
            den_sb = consts.tile([HN, 1], F32)
            ctx_sb = consts.tile([HN, HID], F32)

            # ---- phase 1: scores -> exp -> transposed p~ ----------------
            # s_ps[hn, s] accumulates QSCALE * (q~ . hs) over 16 DoubleRow
            # fp8 matmuls (256 contraction rows each), plus one k=1 bf16
            # matmul adding the per-position mask bias via ones[hn] x mb[s].
            with (
                tc.tile_pool(name="sps", bufs=2, space="PSUM") as sps,
                tc.tile_pool(name="tps", bufs=2, space="PSUM") as tps,
                tc.tile_pool(name="pcb", bufs=2) as pcb,
            ):
                for ch in range(NCH):
                    hsT_blk = hstp.tile([128, NK, CHW], F8, tag="hst")
                    nc.sync.dma_start(
                        out=hsT_blk,
                        in_=hsT8[:, ch * CHW : (ch + 1) * CHW].rearrange(
                            "(ko ki) n -> ki ko n", ki=128
                        ),
                    )
                    s_ps = sps.tile([128, CHW], F32, tag="s")
                    for t in range(NK // 2):
                        nc.tensor.matmul(
                            s_ps[0:HN, :],
                            qe_sb[:, 2 * t : 2 * t + 2, :],
                            hsT_blk[:, 2 * t : 2 * t + 2, :],
                            start=(t == 0),
                            stop=False,
                            perf_mode=mybir.MatmulPerfMode.DoubleRow,
                        )
                    nc.tensor.matmul(
                        s_ps[0:HN, :],
                        ones_sb,
                        mb_sb[:, ch * CHW : (ch + 1) * CHW],
                        start=False,
                        stop=True,
                        skip_group_check=True,
                    )
                    p_blk = pcb.tile([HN, CHW], BF16, tag="p")
                    nc.scalar.activation(
                        out=p_blk,
                        in_=s_ps[0:HN, :],
                        func=mybir.ActivationFunctionType.Exp,
                        scale=EXPSCALE,
                        accum_out=den4[:, ch : ch + 1],
                    )
                    for j in range(CHW // 128):
                        t_ps = tps.tile([128, 1024], BF16, tag="t")
                        nc.tensor.transpose(
                            t_ps[:, 0:HN], p_blk[:, j * 128 : (j + 1) * 128], ident
                        )
                        nc.vector.tensor_copy(
                            out=pT_sb[:, ch * (CHW // 128) + j, :],
                            in_=t_ps[:, 0:HN],
                        )
                nc.vector.tensor_reduce(
                    out=den_sb,
                    in_=den4,
                    axis=mybir.AxisListType.X,
                    op=mybir.AluOpType.add,
                )

            # ---- phase 2: ctx = p~ @ hs, streaming hs row-tiles ---------
            with tc.tile_pool(name="cps", bufs=1, space="PSUM") as cps:
                ctx_ps = [
                    cps.tile([128, 512], F32, tag=f"c{cg}", name=f"ctx{cg}")
                    for cg in range(NCG)
                ]
                for st in range(NST):
                    hs_blk = hsp.tile([128, HID], BF16, tag="hs")
                    nc.sync.dma_start(
                        out=hs_blk, in_=hs16[st * 128 : (st + 1) * 128, :]
                    )
                    for cg in range(NCG):
                        nc.tensor.matmul(
                            ctx_ps[cg][0:HN, :],
                            pT_sb[:, st, :],
                            hs_blk[:, cg * 512 : (cg + 1) * 512],
                            start=(st == 0),
                            stop=(st == NST - 1),
                        )
                for cg in range(NCG):
                    nc.vector.tensor_copy(
                        out=ctx_sb[:, cg * 512 : (cg + 1) * 512],
                        in_=ctx_ps[cg][0:HN, :],
                    )
            nc.sync.dma_start(out=ctx_out, in_=ctx_sb)
            nc.sync.dma_start(out=den_out, in_=den_sb)

    nc.compile()
    return nc


def _get_module():
    key = (HID, S)
    if key not in _cache:
        _cache[key] = _build_module()
    return _cache[key]


def _prep_in_maps(hs, mask, ms, Wq, Wk, Wv, Wo):
    """Shard the full inputs into 8 per-core input maps (host-side)."""
    # qe[h*8+n, :] = (Q[n, h*64:(h+1)*64] @ Wk[h*64:(h+1)*64, :]) * QSCALE
    Q = ms @ Wq.T  # [slots, BD]
    Qh = Q.reshape(SLOTS, HEADS, HD)
    Wkh = Wk.reshape(HEADS, HD, HID)
    qe = np.einsum("nhd,hdi->hni", Qh, Wkh, optimize=True).reshape(HN, HID)
    qe = (qe * QSCALE).astype(np.float32)
    # pre-tile for a contiguous DMA: row ki holds [ko, hn] blocks
    qe8_host = np.ascontiguousarray(
        qe.T.reshape(NK, 128, HN).transpose(1, 0, 2).reshape(128, NK * HN)
    ).astype(npf8)

    mbias = np.where(mask == 0, np.float32(MASK_NEG), np.float32(0.0)).astype(
        npbf16
    )  # [B, S]

    in_maps = []
    for b in range(B):
        hsT_b = np.ascontiguousarray(hs[b].T)  # [HID, S] f32
        for g in range(HALVES):
            sl = slice(g * SH, (g + 1) * SH)
            in_maps.append(
                {
                    "hsT8": hsT_b[:, sl].astype(npf8),
                    "hs16": hs[b, sl, :].astype(npbf16),
                    "qe8": qe8_host,
                    "mbT": np.ascontiguousarray(mbias[b, sl]).reshape(1, SH),
                }
            )
    return in_maps


def _host_finish(res, Wv, Wo):
    """Combine per-core ctx/den partials and apply the tiny projections."""
    Wvh = Wv.reshape(HEADS, HD, HID)  # [h, d, i]
    y = np.empty((B, SLOTS, HID), np.float32)
    for b in range(B):
        r0 = res[HALVES * b]
        r1 = res[HALVES * b + 1]
        numer = r0["ctx_out"] + r1["ctx_out"]  # [HN, HID]
        den = r0["den_out"] + r1["den_out"]  # [HN, 1]
        ctx = (numer / den).reshape(HEADS, SLOTS, HID)  # [h, n, i]
        z = np.einsum("hni,hdi->nhd", ctx, Wvh, optimize=True)  # [n, h, d]
        y[b] = z.reshape(SLOTS, BD) @ Wo.T
    return y


def time_device(inputs_np, reps=8):
    """Dev-only helper (not used by grading): time repeated NEFF executions
    with inputs resident on device. Mirrors bass2jax.run_bass_via_pjrt's
    multi-core path; each wall time includes one axon execute round-trip."""
    import time

    import jax
    from jax.experimental.shard_map import shard_map
    from jax.sharding import Mesh, NamedSharding, PartitionSpec

    import concourse.mybir as mybir_
    from concourse import bass2jax

    nc = _get_module()
    in_maps = _prep_in_maps(
        np.asarray(inputs_np["hidden_states"], np.float32),
        np.asarray(inputs_np["attention_mask"]),
        np.asarray(inputs_np["memory_slots"], np.float32),
        np.asarray(inputs_np["Wq"], np.float32),
        np.asarray(inputs_np["Wk"], np.float32),
        np.asarray(inputs_np["Wv"], np.float32),
        np.asarray(inputs_np["Wo"], np.float32),
    )
    bass2jax.install_neuronx_cc_hook()

    in_names, out_names, out_avals, zero_outs = [], [], [], []
    has_partition = False
    for alloc in nc.m.functions[0].allocations:
        if not isinstance(alloc, mybir_.MemoryLocationSet):
            continue
        name = alloc.memorylocations[0].name
        if alloc.kind == "ExternalInput":
            if name == "partition_id":
                has_partition = True
                continue
            in_names.append(name)
        elif alloc.kind == "ExternalOutput":
            out_names.append(name)
            shape = tuple(alloc.tensor_shape)
            dtype = mybir_.dt.np(alloc.dtype)
            out_avals.append(jax.core.ShapedArray(shape, dtype))
            zero_outs.append(np.zeros(shape, dtype))
    n_params = len(in_names)
    n_outs = len(out_avals)
    # Operand order must match run_bass_via_pjrt: inputs, donated output
    # zeros, then partition-id LAST (neuronx_cc_hook checks operands[:-1]
    # are jit parameters 0..N-1).
    all_names = in_names + out_names + (["partition_id"] if has_partition else [])

    def _body(*args):
        operands = list(args)
        if has_partition:
            operands.append(bass2jax.partition_id_tensor())
        outs = bass2jax._bass_exec_p.bind(
            *operands,
            out_avals=tuple(out_avals),
            in_names=tuple(all_names),
            out_names=tuple(out_names),
            lowering_input_output_aliases=(),
            sim_require_finite=True,
            sim_require_nnan=True,
            nc=nc,
        )
        return tuple(outs)

    devices = jax.devices()[:N_CORES]
    mesh = Mesh(np.asarray(devices), ("core",))
    spec = PartitionSpec("core")
    sharded = jax.jit(
        shard_map(
            _body,
            mesh=mesh,
            in_specs=(spec,) * (n_params + n_outs),
            out_specs=(spec,) * n_outs,
            check_rep=False,
        ),
        donate_argnums=tuple(range(n_params, n_params + n_outs)),
        keep_unused=True,
    )
    concat_in = [
        np.concatenate([np.asarray(in_maps[c][nm]) for c in range(N_CORES)], axis=0)
        for nm in in_names
    ]
    sh = NamedSharding(mesh, spec)
    dev_in = [jax.device_put(a, sh) for a in concat_in]
    jax.block_until_ready(dev_in)

    times = []
    for _ in range(reps):
        zeros = [np.zeros((N_CORES * z.shape[0], *z.shape[1:]), z.dtype)
                 for z in zero_outs]
        dz = [jax.device_put(z, sh) for z in zeros]
        jax.block_until_ready(dz)
        t0 = time.perf_counter()
        out = sharded(*dev_in, *dz)
        jax.block_until_ready(out)
        times.append(time.perf_counter() - t0)
    return times


def kernel(hidden_states, attention_mask, memory_slots, Wq, Wk, Wv, Wo):
    global LAST_RESULT
    hs = np.asarray(hidden_states, dtype=np.float32)
    mask = np.asarray(attention_mask)
    ms = np.asarray(memory_slots, dtype=np.float32)
    Wq = np.asarray(Wq, dtype=np.float32)
    Wk = np.asarray(Wk, dtype=np.float32)
    Wv = np.asarray(Wv, dtype=np.float32)
    Wo = np.asarray(Wo, dtype=np.float32)

    nc = _get_module()
    in_maps = _prep_in_maps(hs, mask, ms, Wq, Wk, Wv, Wo)

    kwargs = {}
    if TRACE:
        kwargs = {"trace": True}
        if TRACE_CORES is not None:
            kwargs["trace_cores"] = TRACE_CORES
    res = run_bass_kernel_spmd(nc, in_maps, core_ids=list(range(N_CORES)), **kwargs)
    LAST_RESULT = res

    y = _host_finish(res.results, Wv, Wo)
    return np.ascontiguousarray(y.astype(np.float32))


# revision 5
# speedup vs baseline: 1.2096x; 1.2096x over previous
"""Trainium2 Bass kernel: memory-slot cross-attention (nn_LocalConstructorMulti).

Algebraic restructuring vs the reference:
    scores[b,h,n,s] = (Q[n,h,:] . K[b,s,h,:]) / 8
                    = hs[b,s,:] . qe[h*8+n,:]        with qe = fold(Wk, Wq, ms)
    out[b,n,h,:]    = Wv_h @ ctx[b,h*8+n,:]          with ctx = attn-weighted
                                                     sum of hidden states
    y[b,n,:]        = sum_h Wo_h (Wv_h ctx_hn)

So the device only computes, per (batch, seq-half) core:
    Phase 1: s[hn, s]  = qe8.T @ hsT8   (fp8, DoubleRow)   [64, cap]
             p~[hn, s] = exp(s * EXPSCALE + maskbias)      (unnormalized)
    Phase 2: ctx[hn,:] = p~ @ hs        (bf16)             [64, 4096]
             den[hn]   = sum_s p~                          (ACT accum_out)
The tiny Wv/Wo projections and the 1/den normalization happen on host
(linear ops commute with the attention sum; den differs per (h,n) so the
normalization must precede the Wo mix, after summing the two seq-halves).

This removes the big K/V projections entirely and the kernel becomes
DMA-bound on reading hs once in bf16 plus once transposed in fp8e4m3
(fp8 is accurate enough for scores because the per-element quantization
noise averages over the 4096-long contraction).

Masked tokens contribute exactly zero (p~ = exp(-1e9 * EXPSCALE) = 0),
so the host drops them before sharding: only unmasked tokens are shipped,
padded to a fixed per-core capacity (multiple of 256, default 1280 which
covers any ~50%-dense 4096-token mask split two ways).  Padding rows get
hs = 0 and bias -1e9, i.e. they also contribute exactly zero.

Sharding: 8 cores = 4 batches x 2 halves of each batch's unmasked-token
list. Host sums the halves' unnormalized ctx/den, normalizes, applies
Wv/Wo.
"""

import sys

if "/opt/trn_rl_repo" not in sys.path:
    sys.path.insert(0, "/opt/trn_rl_repo")

import ml_dtypes
import numpy as np

import concourse.bass as bass  # noqa: F401  (AP helpers)
import concourse.mybir as mybir
import concourse.tile as tile
from concourse import bacc
from concourse.bass_utils import run_bass_kernel_spmd
from concourse.masks import make_identity

BF16 = mybir.dt.bfloat16
F32 = mybir.dt.float32
F8 = mybir.dt.float8e4
npbf16 = ml_dtypes.bfloat16
npf8 = ml_dtypes.float8_e4m3

B, S, HID = 4, 4096, 4096
SLOTS, HEADS, BD = 8, 8, 512
HD = BD // HEADS  # 64
N_CORES = 8
HALVES = 2
HN = HEADS * SLOTS  # 64 (head, slot) pairs
NK = HID // 128  # 32 contraction k-tiles
NCG = HID // 512  # 8 ctx col groups (PSUM banks)
QSCALE = 512.0  # qe pre-scale so fp8 values sit in the normal range
SCALE = 1.0 / float(np.sqrt(HD))
EXPSCALE = 1.0 / (QSCALE / SCALE)  # undo QSCALE, apply 1/sqrt(HD)
MASK_NEG = -1.0e9  # after *EXPSCALE -> -244k -> exp -> 0
DEF_CAP = 1280  # per-core token capacity (multiple of 256)

# test.py can flip this to capture an NTFF profile; harness never touches it.
TRACE = False
TRACE_CORES = None
LAST_RESULT = None

_cache = {}


def _chunks(cap):
    """Score chunk widths: 512s then an optional 256 (PSUM fp32 bank = 512)."""
    out = [512] * (cap // 512)
    if cap % 512:
        out.append(256)
    assert sum(out) == cap
    return out


def _build_module(cap):
    """Emit + compile the single-core Bass module (same NEFF on all cores)."""
    assert cap % 256 == 0
    nst = cap // 128
    widths = _chunks(cap)
    nch = len(widths)

    nc = bacc.Bacc("TRN2", target_bir_lowering=False, debug=False, num_devices=N_CORES)

    hsT8 = nc.dram_tensor("hsT8", [HID, cap], F8, kind="ExternalInput").ap()
    hs16 = nc.dram_tensor("hs16", [cap, HID], BF16, kind="ExternalInput").ap()
    qe8 = nc.dram_tensor("qe8", [128, NK * HN], F8, kind="ExternalInput").ap()
    mbT = nc.dram_tensor("mbT", [1, cap], BF16, kind="ExternalInput").ap()
    ctx_out = nc.dram_tensor("ctx_out", [HN, HID], BF16, kind="ExternalOutput").ap()
    den_out = nc.dram_tensor("den_out", [HN, 1], F32, kind="ExternalOutput").ap()

    with tile.TileContext(nc) as tc:
        with (
            tc.tile_pool(name="consts", bufs=1) as consts,
            tc.tile_pool(name="hsp", bufs=6) as hsp,
        ):
            # ---- resident tensors; hsT chunk 0 is DMA'd first ------------
            hsT_sb = consts.tile([128, NK, cap], F8)
            qe_sb = consts.tile([128, NK, HN], F8)
            mb_sb = consts.tile([1, cap], BF16)
            off = 0
            for ch, w in enumerate(widths):
                nc.sync.dma_start(
                    out=hsT_sb[:, :, off : off + w],
                    in_=hsT8[:, off : off + w].rearrange(
                        "(ko ki) n -> ki ko n", ki=128
                    ),
                )
                if ch == 0:
                    nc.sync.dma_start(
                        out=qe_sb, in_=qe8.rearrange("p (a b) -> p a b", a=NK)
                    )
                    nc.sync.dma_start(out=mb_sb, in_=mbT)
                off += w
            ones_sb = consts.tile([1, HN], BF16)
            nc.vector.memset(ones_sb, 1.0)
            ident = consts.tile([HN, HN], BF16)
            make_identity(nc, ident)
            pT_sb = consts.tile([128, nst, HN], BF16)
            denc = consts.tile([HN, nch], F32)
            den_sb = consts.tile([HN, 1], F32)
            ctx_sb = consts.tile([HN, HID], BF16)

            # ---- phase 1: scores -> exp -> transposed p~ ----------------
            # s_ps[hn, s] accumulates QSCALE * (q~ . hs) over 16 DoubleRow
            # fp8 matmuls (256 contraction rows each), plus one k=1 bf16
            # matmul adding the per-position mask bias via ones[hn] x mb[s].
            with (
                tc.tile_pool(name="sps", bufs=2, space="PSUM") as sps,
                tc.tile_pool(name="tps", bufs=2, space="PSUM") as tps,
                tc.tile_pool(name="pcb", bufs=2) as pcb,
            ):
                off = 0
                for ch, w in enumerate(widths):
                    s_ps = sps.tile([128, 512], F32, tag="s")
                    for t in range(NK // 2):
                        nc.tensor.matmul(
                            s_ps[0:HN, 0:w],
                            qe_sb[:, 2 * t : 2 * t + 2, :],
                            hsT_sb[:, 2 * t : 2 * t + 2, off : off + w],
                            start=(t == 0),
                            stop=False,
                            perf_mode=mybir.MatmulPerfMode.DoubleRow,
                        )
                    nc.tensor.matmul(
                        s_ps[0:HN, 0:w],
                        ones_sb,
                        mb_sb[:, off : off + w],
                        start=False,
                        stop=True,
                        skip_group_check=True,
                    )
                    p_blk = pcb.tile([HN, 512], BF16, tag="p")
                    nc.scalar.activation(
                        out=p_blk[:, 0:w],
                        in_=s_ps[0:HN, 0:w],
                        func=mybir.ActivationFunctionType.Exp,
                        scale=EXPSCALE,
                        accum_out=denc[:, ch : ch + 1],
                    )
                    for j in range(w // 128):
                        t_ps = tps.tile([128, 1024], BF16, tag="t")
                        nc.tensor.transpose(
                            t_ps[:, 0:HN], p_blk[:, j * 128 : (j + 1) * 128], ident
                        )
                        nc.vector.tensor_copy(
                            out=pT_sb[:, off // 128 + j, :],
                            in_=t_ps[:, 0:HN],
                        )
                    off += w
                nc.vector.tensor_reduce(
                    out=den_sb,
                    in_=denc,
                    axis=mybir.AxisListType.X,
                    op=mybir.AluOpType.add,
                )
                nc.sync.dma_start(out=den_out, in_=den_sb)

            # ---- phase 2: ctx = p~ @ hs, streaming hs row-tiles ---------
            with tc.tile_pool(name="cps", bufs=1, space="PSUM") as cps:
                ctx_ps = [
                    cps.tile([128, 512], F32, tag=f"c{cg}", name=f"ctx{cg}")
                    for cg in range(NCG)
                ]
                for st in range(nst):
                    hs_blk = hsp.tile([128, HID], BF16, tag="hs")
                    nc.sync.dma_start(
                        out=hs_blk, in_=hs16[st * 128 : (st + 1) * 128, :]
                    )
                    for cg in range(NCG):
                        nc.tensor.matmul(
                            ctx_ps[cg][0:HN, :],
                            pT_sb[:, st, :],
                            hs_blk[:, cg * 512 : (cg + 1) * 512],
                            start=(st == 0),
                            stop=(st == nst - 1),
                        )
                # drain + ship per bank, alternating ACT/DVE so the copies
                # overlap; each 512-col group DMAs out as soon as it lands.
                for cg in range(NCG):
                    sl = slice(cg * 512, (cg + 1) * 512)
                    if cg % 2 == 0:
                        nc.scalar.copy(out=ctx_sb[:, sl], in_=ctx_ps[cg][0:HN, :])
                    else:
                        nc.vector.tensor_copy(
                            out=ctx_sb[:, sl], in_=ctx_ps[cg][0:HN, :]
                        )
                    nc.sync.dma_start(out=ctx_out[:, sl], in_=ctx_sb[:, sl])

    nc.compile()
    return nc


def _get_module(cap=DEF_CAP):
    if cap not in _cache:
        _cache[cap] = _build_module(cap)
    return _cache[cap]


def _prep_in_maps(hs, mask, ms, Wq, Wk, Wv, Wo):
    """Compact away masked tokens, shard into 8 per-core input maps."""
    # qe[h*8+n, :] = (Q[n, h*64:(h+1)*64] @ Wk[h*64:(h+1)*64, :]) * QSCALE
    Q = ms @ Wq.T  # [slots, BD]
    Qh = Q.reshape(SLOTS, HEADS, HD)
    Wkh = Wk.reshape(HEADS, HD, HID)
    qe = np.einsum("nhd,hdi->hni", Qh, Wkh, optimize=True).reshape(HN, HID)
    qe = (qe * QSCALE).astype(np.float32)
    # pre-tile for a contiguous DMA: row ki holds [ko, hn] blocks
    qe8_host = np.ascontiguousarray(
        qe.T.reshape(NK, 128, HN).transpose(1, 0, 2).reshape(128, NK * HN)
    ).astype(npf8)

    kept = [np.flatnonzero(mask[b] != 0) for b in range(B)]
    need = max((len(k) + HALVES - 1) // HALVES for k in kept)
    cap = max(DEF_CAP, ((need + 255) // 256) * 256)

    in_maps = []
    for b in range(B):
        idx = kept[b]
        hs_keep = hs[b, idx, :]  # [T, HID] f32
        T = len(idx)
        t0 = (T + 1) // 2
        for g, gsl in enumerate((slice(0, t0), slice(t0, T))):
            part = hs_keep[gsl]
            t = part.shape[0]
            hs16 = np.zeros((cap, HID), npbf16)
            hs16[:t] = part.astype(npbf16)
            hsT8 = np.zeros((HID, cap), npf8)
            hsT8[:, :t] = part.T.astype(npf8)
            mb = np.full((1, cap), np.float32(MASK_NEG), npbf16)
            mb[0, :t] = npbf16(0.0)
            in_maps.append(
                {"hsT8": hsT8, "hs16": hs16, "qe8": qe8_host, "mbT": mb}
            )
    return in_maps, cap


def _host_finish(res, Wv, Wo):
    """Combine per-core ctx/den partials and apply the tiny projections."""
    Wvh = Wv.reshape(HEADS, HD, HID)  # [h, d, i]
    y = np.empty((B, SLOTS, HID), np.float32)
    for b in range(B):
        r0 = res[HALVES * b]
        r1 = res[HALVES * b + 1]
        numer = r0["ctx_out"].astype(np.float32) + r1["ctx_out"].astype(np.float32)
        den = r0["den_out"] + r1["den_out"]  # [HN, 1]
        ctx = (numer / den).reshape(HEADS, SLOTS, HID)  # [h, n, i]
        z = np.einsum("hni,hdi->nhd", ctx, Wvh, optimize=True)  # [n, h, d]
        y[b] = z.reshape(SLOTS, BD) @ Wo.T
    return y


def time_device(inputs_np, reps=8):
    """Dev-only helper (not used by grading): time repeated NEFF executions
    with inputs resident on device. Mirrors bass2jax.run_bass_via_pjrt's
    multi-core path; each wall time includes one axon execute round-trip."""
    import time

    import jax
    from jax.experimental.shard_map import shard_map
    from jax.sharding import Mesh, NamedSharding, PartitionSpec

    import concourse.mybir as mybir_
    from concourse import bass2jax

    in_maps, cap = _prep_in_maps(
        np.asarray(inputs_np["hidden_states"], np.float32),
        np.asarray(inputs_np["attention_mask"]),
        np.asarray(inputs_np["memory_slots"], np.float32),
        np.asarray(inputs_np["Wq"], np.float32),
        np.asarray(inputs_np["Wk"], np.float32),
        np.asarray(inputs_np["Wv"], np.float32),
        np.asarray(inputs_np["Wo"], np.float32),
    )
    nc = _get_module(cap)
    bass2jax.install_neuronx_cc_hook()

    in_names, out_names, out_avals, zero_outs = [], [], [], []
    has_partition = False
    for alloc in nc.m.functions[0].allocations:
        if not isinstance(alloc, mybir_.MemoryLocationSet):
            continue
        name = alloc.memorylocations[0].name
        if alloc.kind == "ExternalInput":
            if name == "partition_id":
                has_partition = True
                continue
            in_names.append(name)
        elif alloc.kind == "ExternalOutput":
            out_names.append(name)
            shape = tuple(alloc.tensor_shape)
            dtype = mybir_.dt.np(alloc.dtype)
            out_avals.append(jax.core.ShapedArray(shape, dtype))
            zero_outs.append(np.zeros(shape, dtype))
    n_params = len(in_names)
    n_outs = len(out_avals)
    # Operand order must match run_bass_via_pjrt: inputs, donated output
    # zeros, then partition-id LAST (neuronx_cc_hook checks operands[:-1]
    # are jit parameters 0..N-1).
    all_names = in_names + out_names + (["partition_id"] if has_partition else [])

    def _body(*args):
        operands = list(args)
        if has_partition:
            operands.append(bass2jax.partition_id_tensor())
        outs = bass2jax._bass_exec_p.bind(
            *operands,
            out_avals=tuple(out_avals),
            in_names=tuple(all_names),
            out_names=tuple(out_names),
            lowering_input_output_aliases=(),
            sim_require_finite=True,
            sim_require_nnan=True,
            nc=nc,
        )
        return tuple(outs)

    devices = jax.devices()[:N_CORES]
    mesh = Mesh(np.asarray(devices), ("core",))
    spec = PartitionSpec("core")
    sharded = jax.jit(
        shard_map(
            _body,
            mesh=mesh,
            in_specs=(spec,) * (n_params + n_outs),
            out_specs=(spec,) * n_outs,
            check_rep=False,
        ),
        donate_argnums=tuple(range(n_params, n_params + n_outs)),
        keep_unused=True,
    )
    concat_in = [
        np.concatenate([np.asarray(in_maps[c][nm]) for c in range(N_CORES)], axis=0)
        for nm in in_names
    ]
    sh = NamedSharding(mesh, spec)
    dev_in = [jax.device_put(a, sh) for a in concat_in]
    jax.block_until_ready(dev_in)

    times = []
    for _ in range(reps):
        zeros = [np.zeros((N_CORES * z.shape[0], *z.shape[1:]), z.dtype)
                 for z in zero_outs]
        dz = [jax.device_put(z, sh) for z in zeros]
        jax.block_until_ready(dz)
        t0 = time.perf_counter()
        out = sharded(*dev_in, *dz)
        jax.block_until_ready(out)
        times.append(time.perf_counter() - t0)
    return times


def kernel(hidden_states, attention_mask, memory_slots, Wq, Wk, Wv, Wo):
    global LAST_RESULT
    hs = np.asarray(hidden_states, dtype=np.float32)
    mask = np.asarray(attention_mask)
    ms = np.asarray(memory_slots, dtype=np.float32)
    Wq = np.asarray(Wq, dtype=np.float32)
    Wk = np.asarray(Wk, dtype=np.float32)
    Wv = np.asarray(Wv, dtype=np.float32)
    Wo = np.asarray(Wo, dtype=np.float32)

    in_maps, cap = _prep_in_maps(hs, mask, ms, Wq, Wk, Wv, Wo)
    nc = _get_module(cap)

    kwargs = {}
    if TRACE:
        kwargs = {"trace": True}
        if TRACE_CORES is not None:
            kwargs["trace_cores"] = TRACE_CORES
    res = run_bass_kernel_spmd(nc, in_maps, core_ids=list(range(N_CORES)), **kwargs)
    LAST_RESULT = res

    y = _host_finish(res.results, Wv, Wo)
    return np.ascontiguousarray(y.astype(np.float32))


# revision 10
# speedup vs baseline: 24.2225x; 20.0245x over previous
"""Trainium2 Bass kernel: memory-slot cross-attention (nn_LocalConstructorMulti).

Algebraic restructuring vs the reference:
    scores[b,h,n,s] = (Q[n,h,:] . K[b,s,h,:]) / 8
                    = hs[b,s,:] . qe[h*8+n,:]        with qe = fold(Wk, Wq, ms)
    out[b,n,h,:]    = Wv_h @ ctx[b,h*8+n,:]          with ctx = attn-weighted
                                                     sum of hidden states
    y[b,n,:]        = sum_h Wo_h (Wv_h ctx_hn)

So the device only computes, per (batch, seq-half) core:
    Phase 1: s[hn, s]  = qe8.T @ hsT8   (fp8, DoubleRow)   [64, cap]
             p~[hn, s] = exp(s * EXPSCALE + maskbias)      (unnormalized)
    Phase 2: ctx[hn,:] = p~ @ hs        (bf16)             [64, 4096]
             den[hn]   = sum_s p~                          (ACT accum_out)
The tiny Wv/Wo projections and the 1/den normalization happen on host
(linear ops commute with the attention sum; den differs per (h,n) so the
normalization must precede the Wo mix, after summing the two seq-halves).

This removes the big K/V projections entirely and the kernel becomes
DMA-bound on reading hs once in bf16 plus once transposed in fp8e4m3
(fp8 is accurate enough for scores because the per-element quantization
noise averages over the 4096-long contraction).

Masked tokens contribute exactly zero (p~ = exp(-1e9 * EXPSCALE) = 0),
so the host drops them before sharding: only unmasked tokens are shipped,
padded to a fixed per-core capacity (multiple of 256, default 1280 which
covers any ~50%-dense 4096-token mask split two ways).  Padding rows get
hs = 0 and bias -1e9, i.e. they also contribute exactly zero.

Sharding: 8 cores = 4 batches x 2 halves of each batch's unmasked-token
list. Host sums the halves' unnormalized ctx/den, normalizes, applies
Wv/Wo.
"""

import sys

if "/opt/trn_rl_repo" not in sys.path:
    sys.path.insert(0, "/opt/trn_rl_repo")

import ml_dtypes
import numpy as np

import concourse.bass as bass  # noqa: F401  (AP helpers)
import concourse.mybir as mybir
import concourse.tile as tile
from concourse import bacc
from concourse.bass_utils import run_bass_kernel_spmd
from concourse.masks import make_identity

BF16 = mybir.dt.bfloat16
F32 = mybir.dt.float32
F8 = mybir.dt.float8e4
npbf16 = ml_dtypes.bfloat16
npf8 = ml_dtypes.float8_e4m3

B, S, HID = 4, 4096, 4096
SLOTS, HEADS, BD = 8, 8, 512
HD = BD // HEADS  # 64
N_CORES = 8
HALVES = 2
HN = HEADS * SLOTS  # 64 (head, slot) pairs
NK = HID // 128  # 32 contraction k-tiles
NCG = HID // 512  # 8 ctx col groups (PSUM banks)
QSCALE = 512.0  # qe pre-scale so fp8 values sit in the normal range
SCALE = 1.0 / float(np.sqrt(HD))
EXPSCALE = 1.0 / (QSCALE / SCALE)  # undo QSCALE, apply 1/sqrt(HD)
MASK_NEG = -1.0e9  # after *EXPSCALE -> -244k -> exp -> 0
# Per-core token capacity (multiple of 128). 1152*2 = 2304 unmasked tokens
# per batch = mean + 8 sigma for a Binomial(4096, 1/2) mask — never exceeded
# in practice; larger masks just trigger a one-time recompile at higher cap.
DEF_CAP = 1152

# test.py can flip this to capture an NTFF profile; harness never touches it.
TRACE = False
TRACE_CORES = None
LAST_RESULT = None

_cache = {}


def _chunks(cap):
    """Score chunk widths: 512s then an optional remainder (PSUM bank = 512)."""
    out = [512] * (cap // 512)
    if cap % 512:
        out.append(cap % 512)
    assert sum(out) == cap
    return out


def _build_module(cap):
    """Emit + compile the single-core Bass module (same NEFF on all cores)."""
    assert cap % 128 == 0
    nst = cap // 128
    widths = _chunks(cap)
    nch = len(widths)

    nc = bacc.Bacc("TRN2", target_bir_lowering=False, debug=False, num_devices=N_CORES)

    hsT8 = nc.dram_tensor("hsT8", [HID, cap], F8, kind="ExternalInput").ap()
    hs16 = nc.dram_tensor("hs16", [cap, HID], BF16, kind="ExternalInput").ap()
    qe8 = nc.dram_tensor("qe8", [128, NK * HN], F8, kind="ExternalInput").ap()
    mbT = nc.dram_tensor("mbT", [1, cap], BF16, kind="ExternalInput").ap()
    ctx_out = nc.dram_tensor("ctx_out", [HN, HID], BF16, kind="ExternalOutput").ap()
    den_out = nc.dram_tensor("den_out", [HN, 1], F32, kind="ExternalOutput").ap()

    with tile.TileContext(nc) as tc:
        with (
            tc.tile_pool(name="consts", bufs=1) as consts,
            tc.tile_pool(name="hsp", bufs=6) as hsp,
        ):
            # ---- resident tensors; hsT chunk 0 is DMA'd first ------------
            hsT_sb = consts.tile([128, NK, cap], F8)
            qe_sb = consts.tile([128, NK, HN], F8)
            mb_sb = consts.tile([1, cap], BF16)
            off = 0
            for ch, w in enumerate(widths):
                nc.sync.dma_start(
                    out=hsT_sb[:, :, off : off + w],
                    in_=hsT8[:, off : off + w].rearrange(
                        "(ko ki) n -> ki ko n", ki=128
                    ),
                )
                if ch == 0:
                    nc.sync.dma_start(
                        out=qe_sb, in_=qe8.rearrange("p (a b) -> p a b", a=NK)
                    )
                    nc.sync.dma_start(out=mb_sb, in_=mbT)
                off += w
            ones_sb = consts.tile([1, HN], BF16)
            nc.vector.memset(ones_sb, 1.0)
            ident = consts.tile([HN, HN], BF16)
            make_identity(nc, ident)
            pT_sb = consts.tile([128, nst, HN], BF16)
            denc = consts.tile([HN, nch], F32)
            den_sb = consts.tile([HN, 1], F32)
            ctx_sb = consts.tile([HN, HID], BF16)

            # ---- phase 1: scores -> exp -> transposed p~ ----------------
            # s_ps[hn, s] accumulates QSCALE * (q~ . hs) over 16 DoubleRow
            # fp8 matmuls (256 contraction rows each), plus one k=1 bf16
            # matmul adding the per-position mask bias via ones[hn] x mb[s].
            with (
                tc.tile_pool(name="sps", bufs=2, space="PSUM") as sps,
                tc.tile_pool(name="tps", bufs=2, space="PSUM") as tps,
                tc.tile_pool(name="pcb", bufs=2) as pcb,
            ):
                off = 0
                for ch, w in enumerate(widths):
                    s_ps = sps.tile([128, 512], F32, tag="s")
                    for t in range(NK // 2):
                        nc.tensor.matmul(
                            s_ps[0:HN, 0:w],
                            qe_sb[:, 2 * t : 2 * t + 2, :],
                            hsT_sb[:, 2 * t : 2 * t + 2, off : off + w],
                            start=(t == 0),
                            stop=False,
                            perf_mode=mybir.MatmulPerfMode.DoubleRow,
                        )
                    nc.tensor.matmul(
                        s_ps[0:HN, 0:w],
                        ones_sb,
                        mb_sb[:, off : off + w],
                        start=False,
                        stop=True,
                        skip_group_check=True,
                    )
                    p_blk = pcb.tile([HN, 512], BF16, tag="p")
                    nc.scalar.activation(
                        out=p_blk[:, 0:w],
                        in_=s_ps[0:HN, 0:w],
                        func=mybir.ActivationFunctionType.Exp,
                        scale=EXPSCALE,
                        accum_out=denc[:, ch : ch + 1],
                    )
                    for j in range(w // 128):
                        t_ps = tps.tile([128, 1024], BF16, tag="t")
                        nc.tensor.transpose(
                            t_ps[:, 0:HN], p_blk[:, j * 128 : (j + 1) * 128], ident
                        )
                        nc.vector.tensor_copy(
                            out=pT_sb[:, off // 128 + j, :],
                            in_=t_ps[:, 0:HN],
                        )
                    off += w
                nc.vector.tensor_reduce(
                    out=den_sb,
                    in_=denc,
                    axis=mybir.AxisListType.X,
                    op=mybir.AluOpType.add,
                )
                nc.sync.dma_start(out=den_out, in_=den_sb)

            # ---- phase 2: ctx = p~ @ hs, streaming hs row-tiles ---------
            with tc.tile_pool(name="cps", bufs=1, space="PSUM") as cps:
                ctx_ps = [
                    cps.tile([128, 512], F32, tag=f"c{cg}", name=f"ctx{cg}")
                    for cg in range(NCG)
                ]
                for st in range(nst):
                    hs_blk = hsp.tile([128, HID], BF16, tag="hs")
                    # two column-halves so the first ctx matmuls of this
                    # tile overlap the second half's transfer
                    nc.sync.dma_start(
                        out=hs_blk[:, 0 : HID // 2],
                        in_=hs16[st * 128 : (st + 1) * 128, 0 : HID // 2],
                    )
                    nc.sync.dma_start(
                        out=hs_blk[:, HID // 2 : HID],
                        in_=hs16[st * 128 : (st + 1) * 128, HID // 2 : HID],
                    )
                    for cg in range(NCG):
                        nc.tensor.matmul(
                            ctx_ps[cg][0:HN, :],
                            pT_sb[:, st, :],
                            hs_blk[:, cg * 512 : (cg + 1) * 512],
                            start=(st == 0),
                            stop=(st == nst - 1),
                        )
                # drain + ship per bank, alternating ACT/DVE so the copies
                # overlap; each 512-col group DMAs out as soon as it lands.
                for cg in range(NCG):
                    sl = slice(cg * 512, (cg + 1) * 512)
                    if cg % 2 == 0:
                        nc.scalar.copy(out=ctx_sb[:, sl], in_=ctx_ps[cg][0:HN, :])
                    else:
                        nc.vector.tensor_copy(
                            out=ctx_sb[:, sl], in_=ctx_ps[cg][0:HN, :]
                        )
                    nc.sync.dma_start(out=ctx_out[:, sl], in_=ctx_sb[:, sl])

    nc.compile()
    return nc


def _get_module(cap=DEF_CAP):
    if cap not in _cache:
        _cache[cap] = _build_module(cap)
    return _cache[cap]


def _prep_in_maps(hs, mask, ms, Wq, Wk, Wv, Wo):
    """Compact away masked tokens, shard into 8 per-core input maps."""
    # qe[h*8+n, :] = (Q[n, h*64:(h+1)*64] @ Wk[h*64:(h+1)*64, :]) * QSCALE
    Q = ms @ Wq.T  # [slots, BD]
    Qh = Q.reshape(SLOTS, HEADS, HD)
    Wkh = Wk.reshape(HEADS, HD, HID)
    qe = np.einsum("nhd,hdi->hni", Qh, Wkh, optimize=True).reshape(HN, HID)
    qe = (qe * QSCALE).astype(np.float32)
    # pre-tile for a contiguous DMA: row ki holds [ko, hn] blocks
    qe8_host = np.ascontiguousarray(
        qe.T.reshape(NK, 128, HN).transpose(1, 0, 2).reshape(128, NK * HN)
    ).astype(npf8)

    kept = [np.flatnonzero(mask[b] != 0) for b in range(B)]
    need = max((len(k) + HALVES - 1) // HALVES for k in kept)
    cap = max(DEF_CAP, ((need + 255) // 256) * 256)

    in_maps = []
    for b in range(B):
        idx = kept[b]
        hs_keep = hs[b, idx, :]  # [T, HID] f32
        T = len(idx)
        t0 = (T + 1) // 2
        for g, gsl in enumerate((slice(0, t0), slice(t0, T))):
            part = hs_keep[gsl]
            t = part.shape[0]
            hs16 = np.zeros((cap, HID), npbf16)
            hs16[:t] = part.astype(npbf16)
            hsT8 = np.zeros((HID, cap), npf8)
            hsT8[:, :t] = part.T.astype(npf8)
            mb = np.full((1, cap), np.float32(MASK_NEG), npbf16)
            mb[0, :t] = npbf16(0.0)
            in_maps.append(
                {"hsT8": hsT8, "hs16": hs16, "qe8": qe8_host, "mbT": mb}
            )
    return in_maps, cap


def _host_finish(res, Wv, Wo):
    """Combine per-core ctx/den partials and apply the tiny projections."""
    Wvh = Wv.reshape(HEADS, HD, HID)  # [h, d, i]
    y = np.empty((B, SLOTS, HID), np.float32)
    for b in range(B):
        r0 = res[HALVES * b]
        r1 = res[HALVES * b + 1]
        numer = r0["ctx_out"].astype(np.float32) + r1["ctx_out"].astype(np.float32)
        den = r0["den_out"] + r1["den_out"]  # [HN, 1]
        ctx = (numer / den).reshape(HEADS, SLOTS, HID)  # [h, n, i]
        z = np.einsum("hni,hdi->nhd", ctx, Wvh, optimize=True)  # [n, h, d]
        y[b] = z.reshape(SLOTS, BD) @ Wo.T
    return y


def _timing_setup(inputs_np):
    """Shared scaffolding for the dev-only timing helpers: a compiled
    sharded executable, device-resident inputs, and a fresh-donated-zeros
    factory. Mirrors bass2jax.run_bass_via_pjrt's multi-core path."""
    import jax
    from jax.experimental.shard_map import shard_map
    from jax.sharding import Mesh, NamedSharding, PartitionSpec

    import concourse.mybir as mybir_
    from concourse import bass2jax

    in_maps, cap = _prep_in_maps(
        np.asarray(inputs_np["hidden_states"], np.float32),
        np.asarray(inputs_np["attention_mask"]),
        np.asarray(inputs_np["memory_slots"], np.float32),
        np.asarray(inputs_np["Wq"], np.float32),
        np.asarray(inputs_np["Wk"], np.float32),
        np.asarray(inputs_np["Wv"], np.float32),
        np.asarray(inputs_np["Wo"], np.float32),
    )
    nc = _get_module(cap)
    bass2jax.install_neuronx_cc_hook()

    in_names, out_names, out_avals, zero_outs = [], [], [], []
    has_partition = False
    for alloc in nc.m.functions[0].allocations:
        if not isinstance(alloc, mybir_.MemoryLocationSet):
            continue
        name = alloc.memorylocations[0].name
        if alloc.kind == "ExternalInput":
            if name == "partition_id":
                has_partition = True
                continue
            in_names.append(name)
        elif alloc.kind == "ExternalOutput":
            out_names.append(name)
            shape = tuple(alloc.tensor_shape)
            dtype = mybir_.dt.np(alloc.dtype)
            out_avals.append(jax.core.ShapedArray(shape, dtype))
            zero_outs.append(np.zeros(shape, dtype))
    n_params = len(in_names)
    n_outs = len(out_avals)
    # Operand order must match run_bass_via_pjrt: inputs, donated output
    # zeros, then partition-id LAST (neuronx_cc_hook checks operands[:-1]
    # are jit parameters 0..N-1).
    all_names = in_names + out_names + (["partition_id"] if has_partition else [])

    def _body(*args):
        operands = list(args)
        if has_partition:
            operands.append(bass2jax.partition_id_tensor())
        outs = bass2jax._bass_exec_p.bind(
            *operands,
            out_avals=tuple(out_avals),
            in_names=tuple(all_names),
            out_names=tuple(out_names),
            lowering_input_output_aliases=(),
            sim_require_finite=True,
            sim_require_nnan=True,
            nc=nc,
        )
        return tuple(outs)

    devices = jax.devices()[:N_CORES]
    mesh = Mesh(np.asarray(devices), ("core",))
    spec = PartitionSpec("core")
    sharded = jax.jit(
        shard_map(
            _body,
            mesh=mesh,
            in_specs=(spec,) * (n_params + n_outs),
            out_specs=(spec,) * n_outs,
            check_rep=False,
        ),
        donate_argnums=tuple(range(n_params, n_params + n_outs)),
        keep_unused=True,
    )
    concat_in = [
        np.concatenate([np.asarray(in_maps[c][nm]) for c in range(N_CORES)], axis=0)
        for nm in in_names
    ]
    sh = NamedSharding(mesh, spec)
    dev_in = [jax.device_put(a, sh) for a in concat_in]
    jax.block_until_ready(dev_in)

    def make_dz():
        zeros = [np.zeros((N_CORES * z.shape[0], *z.shape[1:]), z.dtype)
                 for z in zero_outs]
        dz = [jax.device_put(z, sh) for z in zeros]
        jax.block_until_ready(dz)
        return dz

    return sharded, dev_in, make_dz, jax


def time_device(inputs_np, reps=8):
    """Blocking round-trip per-exec wall times (includes full axon RPC
    latency each call)."""
    import time

    sharded, dev_in, make_dz, jax = _timing_setup(inputs_np)
    out = sharded(*dev_in, *make_dz())  # warmup
    jax.block_until_ready(out)
    times = []
    for _ in range(reps):
        dz = make_dz()
        t0 = time.perf_counter()
        out = sharded(*dev_in, *dz)
        jax.block_until_ready(out)
        times.append(time.perf_counter() - t0)
    return times


def time_device_pipelined(inputs_np, depth=64, rounds=2):
    """Per-exec wall time with `depth` executes in flight: amortizes the
    axon round-trip latency, giving the closest wall-clock estimate of
    per-execution device cost available on this tunneled setup."""
    import time

    sharded, dev_in, make_dz, jax = _timing_setup(inputs_np)
    out = sharded(*dev_in, *make_dz())  # warmup
    jax.block_until_ready(out)
    per_exec = []
    for _ in range(rounds):
        dzs = [make_dz() for _ in range(depth)]
        t0 = time.perf_counter()
        outs = [sharded(*dev_in, *dzs[i]) for i in range(depth)]
        jax.block_until_ready(outs)
        per_exec.append((time.perf_counter() - t0) / depth)
    return per_exec


def kernel(hidden_states, attention_mask, memory_slots, Wq, Wk, Wv, Wo):
    global LAST_RESULT
    hs = np.asarray(hidden_states, dtype=np.float32)
    mask = np.asarray(attention_mask)
    ms = np.asarray(memory_slots, dtype=np.float32)
    Wq = np.asarray(Wq, dtype=np.float32)
    Wk = np.asarray(Wk, dtype=np.float32)
    Wv = np.asarray(Wv, dtype=np.float32)
    Wo = np.asarray(Wo, dtype=np.float32)

    in_maps, cap = _prep_in_maps(hs, mask, ms, Wq, Wk, Wv, Wo)
    nc = _get_module(cap)

    kwargs = {}
    if TRACE:
        kwargs = {"trace": True}
        if TRACE_CORES is not None:
            kwargs["trace_cores"] = TRACE_CORES
    res = run_bass_kernel_spmd(nc, in_maps, core_ids=list(range(N_CORES)), **kwargs)
    LAST_RESULT = res

    y = _host_finish(res.results, Wv, Wo)
    return np.ascontiguousarray(y.astype(np.float32))


# revision 29
# speedup vs baseline: 28.1595x; 1.1625x over previous
"""Trainium2 Bass kernel: memory-slot cross-attention (nn_LocalConstructorMulti).

Algebraic restructuring vs the reference:
    scores[b,h,n,s] = (Q[n,h,:] . K[b,s,h,:]) / 8
                    = hs[b,s,:] . qe[h*8+n,:]        with qe = fold(Wk, Wq, ms)
    out[b,n,h,:]    = Wv_h @ ctx[b,h*8+n,:]          with ctx = attn-weighted
                                                     sum of hidden states
    y[b,n,:]        = sum_h Wo_h (Wv_h ctx_hn)

So the device only computes, per (batch, seq-half) core:
    Phase 1: s[hn, s]  = qe8.T @ hsT8   (fp8, DoubleRow)   [64, cap]
             p~[hn, s] = exp(s * EXPSCALE + maskbias)      (unnormalized)
    Phase 2: ctx[hn,:] = p~ @ hs        (bf16)             [64, 4096]
             den[hn]   = sum_s p~                          (ACT accum_out)
The tiny Wv/Wo projections and the 1/den normalization happen on host
(linear ops commute with the attention sum; den differs per (h,n) so the
normalization must precede the Wo mix, after summing the two seq-halves).

This removes the big K/V projections entirely and the kernel becomes
DMA-bound on reading hs once in bf16 plus once transposed in fp8e4m3
(fp8 is accurate enough for scores because the per-element quantization
noise averages over the 4096-long contraction).

Masked tokens contribute exactly zero (p~ = exp(-1e9 * EXPSCALE) = 0),
so the host drops them before sharding: only unmasked tokens are shipped,
padded to a fixed per-core capacity (multiple of 128, default 1152 which
covers any ~50%-dense 4096-token mask split two ways with 8-sigma slack).
Padding rows get hs = 0 and bias -1e9, i.e. they also contribute exactly
zero.

Sharding: 8 cores = 4 batches x 2 halves of each batch's unmasked-token
list. Host sums the halves' unnormalized ctx/den, normalizes, applies
Wv/Wo.
"""

import sys

if "/opt/trn_rl_repo" not in sys.path:
    sys.path.insert(0, "/opt/trn_rl_repo")

import ml_dtypes
import numpy as np

import concourse.bass as bass  # noqa: F401  (AP helpers)
import concourse.mybir as mybir
import concourse.tile as tile
from concourse import bacc
from concourse.bass_utils import run_bass_kernel_spmd
from concourse.masks import make_identity

BF16 = mybir.dt.bfloat16
F32 = mybir.dt.float32
F8 = mybir.dt.float8e4
npbf16 = ml_dtypes.bfloat16
npf8 = ml_dtypes.float8_e4m3

B, S, HID = 4, 4096, 4096
SLOTS, HEADS, BD = 8, 8, 512
HD = BD // HEADS  # 64
N_CORES = 8
HALVES = 2
HN = HEADS * SLOTS  # 64 (head, slot) pairs
NK = HID // 128  # 32 contraction k-tiles
NCG = HID // 512  # 8 ctx col groups (PSUM banks)
QSCALE = 512.0  # qe pre-scale so fp8 values sit in the normal range
SCALE = 1.0 / float(np.sqrt(HD))
EXPSCALE = 1.0 / (QSCALE / SCALE)  # undo QSCALE, apply 1/sqrt(HD)
MASK_NEG = -1.0e9  # after *EXPSCALE -> -244k -> exp -> 0
# Per-core token capacity (multiple of 128). 1152*2 = 2304 unmasked tokens
# per batch = mean + 8 sigma for a Binomial(4096, 1/2) mask — never exceeded
# in practice; larger masks just trigger a one-time recompile at higher cap.
DEF_CAP = 1152

# test.py can flip this to capture an NTFF profile; harness never touches it.
TRACE = False
TRACE_CORES = None
LAST_RESULT = None

_cache = {}


def _chunks(cap):
    """Score chunk widths: 512s then an optional remainder (PSUM bank = 512)."""
    out = [512] * (cap // 512)
    if cap % 512:
        out.append(cap % 512)
    assert sum(out) == cap
    return out


def _build_module(cap):
    """Emit + compile the single-core Bass module (same NEFF on all cores)."""
    assert cap % 128 == 0
    nst = cap // 128
    widths = _chunks(cap)
    nch = len(widths)

    nc = bacc.Bacc("TRN2", target_bir_lowering=False, debug=False, num_devices=N_CORES)

    # hsT8 is pre-tiled on host: chunk ch occupies cols [NK*off, NK*(off+w))
    # with per-partition-contiguous [ko, col] blocks, so each chunk DMA is a
    # single long run per partition (short runs pay a 2x DMA penalty).
    hsT8 = nc.dram_tensor("hsT8", [128, NK * cap], F8, kind="ExternalInput").ap()
    hs16 = nc.dram_tensor("hs16", [cap, HID], BF16, kind="ExternalInput").ap()
    qe8 = nc.dram_tensor("qe8", [128, NK * HN], F8, kind="ExternalInput").ap()
    mbT = nc.dram_tensor("mbT", [1, cap], BF16, kind="ExternalInput").ap()
    ctx_out = nc.dram_tensor("ctx_out", [HN, HID], BF16, kind="ExternalOutput").ap()
    den_out = nc.dram_tensor("den_out", [HN, 1], F32, kind="ExternalOutput").ap()

    with tile.TileContext(nc) as tc:
        with (
            tc.tile_pool(name="consts", bufs=1) as consts,
            tc.tile_pool(name="hsp", bufs=9) as hsp,
        ):
            # ---- resident tensors; hsT chunk 0 is DMA'd first ------------
            hsT_t = [consts.tile([128, NK, w], F8, name=f"hsT{ch}")
                     for ch, w in enumerate(widths)]
            qe_sb = consts.tile([128, NK, HN], F8)
            mb_sb = consts.tile([1, cap], BF16)
            off = 0
            for ch, w in enumerate(widths):
                nc.sync.dma_start(
                    out=hsT_t[ch],
                    in_=hsT8[:, NK * off : NK * (off + w)].rearrange(
                        "p (a b) -> p a b", a=NK
                    ),
                )
                if ch == 0:
                    nc.sync.dma_start(
                        out=qe_sb, in_=qe8.rearrange("p (a b) -> p a b", a=NK)
                    )
                    nc.sync.dma_start(out=mb_sb, in_=mbT)
                off += w
            ones_sb = consts.tile([1, HN], BF16)
            nc.vector.memset(ones_sb, 1.0)
            ident = consts.tile([HN, HN], BF16)
            make_identity(nc, ident)
            pT_sb = consts.tile([128, nst, HN], BF16)
            denc = consts.tile([HN, nch], F32)
            den_sb = consts.tile([HN, 1], F32)
            ctx_sb = consts.tile([HN, HID], BF16)

            # ---- phase 1: scores -> exp -> transposed p~ ----------------
            # s_ps[hn, s] accumulates QSCALE * (q~ . hs) over 16 DoubleRow
            # fp8 matmuls (256 contraction rows each), plus one k=1 bf16
            # matmul adding the per-position mask bias via ones[hn] x mb[s].
            with (
                tc.tile_pool(name="sps", bufs=2, space="PSUM") as sps,
                tc.tile_pool(name="tps", bufs=2, space="PSUM") as tps,
                tc.tile_pool(name="pcb", bufs=2) as pcb,
            ):
                off = 0
                for ch, w in enumerate(widths):
                    s_ps = sps.tile([128, 512], F32, tag="s")
                    for t in range(NK // 2):
                        nc.tensor.matmul(
                            s_ps[0:HN, 0:w],
                            qe_sb[:, 2 * t : 2 * t + 2, :],
                            hsT_t[ch][:, 2 * t : 2 * t + 2, :],
                            start=(t == 0),
                            stop=False,
                            perf_mode=mybir.MatmulPerfMode.DoubleRow,
                        )
                    nc.tensor.matmul(
                        s_ps[0:HN, 0:w],
                        ones_sb,
                        mb_sb[:, off : off + w],
                        start=False,
                        stop=True,
                        skip_group_check=True,
                    )
                    p_blk = pcb.tile([HN, 512], BF16, tag="p")
                    nc.scalar.activation(
                        out=p_blk[:, 0:w],
                        in_=s_ps[0:HN, 0:w],
                        func=mybir.ActivationFunctionType.Exp,
                        scale=EXPSCALE,
                        accum_out=denc[:, ch : ch + 1],
                    )
                    for j in range(w // 128):
                        t_ps = tps.tile([128, 1024], BF16, tag="t")
                        nc.tensor.transpose(
                            t_ps[:, 0:HN], p_blk[:, j * 128 : (j + 1) * 128], ident
                        )
                        nc.vector.tensor_copy(
                            out=pT_sb[:, off // 128 + j, :],
                            in_=t_ps[:, 0:HN],
                        )
                    off += w
                nc.vector.tensor_reduce(
                    out=den_sb,
                    in_=denc,
                    axis=mybir.AxisListType.X,
                    op=mybir.AluOpType.add,
                )
                nc.sync.dma_start(out=den_out, in_=den_sb)

            # ---- phase 2: ctx = p~ @ hs, streaming hs row-tiles ---------
            with tc.tile_pool(name="cps", bufs=1, space="PSUM") as cps:
                ctx_ps = [
                    cps.tile([128, 512], F32, tag=f"c{cg}", name=f"ctx{cg}")
                    for cg in range(NCG)
                ]
                for st in range(nst):
                    hs_blk = hsp.tile([128, HID], BF16, tag="hs")
                    # same SP ring as the hsT stream: ring order preserves
                    # the phase-1-first DMA priority
                    nc.sync.dma_start(
                        out=hs_blk, in_=hs16[st * 128 : (st + 1) * 128, :]
                    )
                    for cg in range(NCG):
                        nc.tensor.matmul(
                            ctx_ps[cg][0:HN, :],
                            pT_sb[:, st, :],
                            hs_blk[:, cg * 512 : (cg + 1) * 512],
                            start=(st == 0),
                            stop=(st == nst - 1),
                        )
                # Drain bank PAIRS per engine (ACT: 0+1, 4+5; DVE: 2+3, 6+7)
                # so each 1024-col output region has a single writer — a DMA
                # spanning two engines' writes raced intermittently on HW.
                # Pair DMAs split across the SP and ACT rings (both idle by
                # now) to halve the per-ring DGE spacing at the tail.
                for pair in range(NCG // 2):
                    eng_act = pair % 2 == 0
                    for cg in (2 * pair, 2 * pair + 1):
                        sl = slice(cg * 512, (cg + 1) * 512)
                        if eng_act:
                            nc.scalar.copy(
                                out=ctx_sb[:, sl], in_=ctx_ps[cg][0:HN, :]
                            )
                        else:
                            nc.vector.tensor_copy(
                                out=ctx_sb[:, sl], in_=ctx_ps[cg][0:HN, :]
                            )
                    osl = slice(2 * pair * 512, (2 * pair + 2) * 512)
                    ring = nc.sync if eng_act else nc.scalar
                    ring.dma_start(out=ctx_out[:, osl], in_=ctx_sb[:, osl])

    nc.compile()
    return nc


def _get_module(cap=DEF_CAP):
    if cap not in _cache:
        _cache[cap] = _build_module(cap)
    return _cache[cap]


def _prep_in_maps(hs, mask, ms, Wq, Wk, Wv, Wo):
    """Compact away masked tokens, shard into 8 per-core input maps."""
    # qe[h*8+n, :] = (Q[n, h*64:(h+1)*64] @ Wk[h*64:(h+1)*64, :]) * QSCALE
    Q = ms @ Wq.T  # [slots, BD]
    Qh = Q.reshape(SLOTS, HEADS, HD)
    Wkh = Wk.reshape(HEADS, HD, HID)
    qe = np.einsum("nhd,hdi->hni", Qh, Wkh, optimize=True).reshape(HN, HID)
    qe = (qe * QSCALE).astype(np.float32)
    # pre-tile for a contiguous DMA: row ki holds [ko, hn] blocks
    qe8_host = np.ascontiguousarray(
        qe.T.reshape(NK, 128, HN).transpose(1, 0, 2).reshape(128, NK * HN)
    ).astype(npf8)

    kept = [np.flatnonzero(mask[b] != 0) for b in range(B)]
    need = max((len(k) + HALVES - 1) // HALVES for k in kept)
    cap = max(DEF_CAP, ((need + 255) // 256) * 256)

    in_maps = []
    for b in range(B):
        idx = kept[b]
        hs_keep = hs[b, idx, :]  # [T, HID] f32
        T = len(idx)
        t0 = (T + 1) // 2
        for g, gsl in enumerate((slice(0, t0), slice(t0, T))):
            part = hs_keep[gsl]
            t = part.shape[0]
            hs16 = np.zeros((cap, HID), npbf16)
            hs16[:t] = part.astype(npbf16)
            hsT = np.zeros((HID, cap), npf8)
            hsT[:, :t] = part.T.astype(npf8)
            # pre-tile per score-chunk: [128, NK*w] blocks, ko-major per row
            hsT8 = np.concatenate(
                [
                    np.ascontiguousarray(
                        hsT[:, o : o + w]
                        .reshape(NK, 128, w)
                        .transpose(1, 0, 2)
                        .reshape(128, NK * w)
                    )
                    for o, w in zip(np.cumsum([0] + _chunks(cap)[:-1]), _chunks(cap))
                ],
                axis=1,
            )
            mb = np.full((1, cap), np.float32(MASK_NEG), npbf16)
            mb[0, :t] = npbf16(0.0)
            in_maps.append(
                {"hsT8": hsT8, "hs16": hs16, "qe8": qe8_host, "mbT": mb}
            )
    return in_maps, cap


def _host_finish(res, Wv, Wo):
    """Combine per-core ctx/den partials and apply the tiny projections."""
    Wvh = Wv.reshape(HEADS, HD, HID)  # [h, d, i]
    y = np.empty((B, SLOTS, HID), np.float32)
    for b in range(B):
        r0 = res[HALVES * b]
        r1 = res[HALVES * b + 1]
        numer = r0["ctx_out"].astype(np.float32) + r1["ctx_out"].astype(np.float32)
        den = r0["den_out"] + r1["den_out"]  # [HN, 1]
        ctx = (numer / den).reshape(HEADS, SLOTS, HID)  # [h, n, i]
        z = np.einsum("hni,hdi->nhd", ctx, Wvh, optimize=True)  # [n, h, d]
        y[b] = z.reshape(SLOTS, BD) @ Wo.T
    return y


def _timing_setup(inputs_np):
    """Shared scaffolding for the dev-only timing helpers: a compiled
    sharded executable, device-resident inputs, and a fresh-donated-zeros
    factory. Mirrors bass2jax.run_bass_via_pjrt's multi-core path."""
    import jax
    from jax.experimental.shard_map import shard_map
    from jax.sharding import Mesh, NamedSharding, PartitionSpec

    import concourse.mybir as mybir_
    from concourse import bass2jax

    in_maps, cap = _prep_in_maps(
        np.asarray(inputs_np["hidden_states"], np.float32),
        np.asarray(inputs_np["attention_mask"]),
        np.asarray(inputs_np["memory_slots"], np.float32),
        np.asarray(inputs_np["Wq"], np.float32),
        np.asarray(inputs_np["Wk"], np.float32),
        np.asarray(inputs_np["Wv"], np.float32),
        np.asarray(inputs_np["Wo"], np.float32),
    )
    nc = _get_module(cap)
    bass2jax.install_neuronx_cc_hook()

    in_names, out_names, out_avals, zero_outs = [], [], [], []
    has_partition = False
    for alloc in nc.m.functions[0].allocations:
        if not isinstance(alloc, mybir_.MemoryLocationSet):
            continue
        name = alloc.memorylocations[0].name
        if alloc.kind == "ExternalInput":
            if name == "partition_id":
                has_partition = True
                continue
            in_names.append(name)
        elif alloc.kind == "ExternalOutput":
            out_names.append(name)
            shape = tuple(alloc.tensor_shape)
            dtype = mybir_.dt.np(alloc.dtype)
            out_avals.append(jax.core.ShapedArray(shape, dtype))
            zero_outs.append(np.zeros(shape, dtype))
    n_params = len(in_names)
    n_outs = len(out_avals)
    # Operand order must match run_bass_via_pjrt: inputs, donated output
    # zeros, then partition-id LAST (neuronx_cc_hook checks operands[:-1]
    # are jit parameters 0..N-1).
    all_names = in_names + out_names + (["partition_id"] if has_partition else [])

    def _body(*args):
        operands = list(args)
        if has_partition:
            operands.append(bass2jax.partition_id_tensor())
        outs = bass2jax._bass_exec_p.bind(
            *operands,
            out_avals=tuple(out_avals),
            in_names=tuple(all_names),
            out_names=tuple(out_names),
            lowering_input_output_aliases=(),
            sim_require_finite=True,
            sim_require_nnan=True,
            nc=nc,
        )
        return tuple(outs)

    devices = jax.devices()[:N_CORES]
    mesh = Mesh(np.asarray(devices), ("core",))
    spec = PartitionSpec("core")
    sharded = jax.jit(
        shard_map(
            _body,
            mesh=mesh,
            in_specs=(spec,) * (n_params + n_outs),
            out_specs=(spec,) * n_outs,
            check_rep=False,
        ),
        donate_argnums=tuple(range(n_params, n_params + n_outs)),
        keep_unused=True,
    )
    concat_in = [
        np.concatenate([np.asarray(in_maps[c][nm]) for c in range(N_CORES)], axis=0)
        for nm in in_names
    ]
    sh = NamedSharding(mesh, spec)
    dev_in = [jax.device_put(a, sh) for a in concat_in]
    jax.block_until_ready(dev_in)

    def make_dz():
        zeros = [np.zeros((N_CORES * z.shape[0], *z.shape[1:]), z.dtype)
                 for z in zero_outs]
        dz = [jax.device_put(z, sh) for z in zeros]
        jax.block_until_ready(dz)
        return dz

    return sharded, dev_in, make_dz, jax


def time_device(inputs_np, reps=8):
    """Blocking round-trip per-exec wall times (includes full axon RPC
    latency each call)."""
    import time

    sharded, dev_in, make_dz, jax = _timing_setup(inputs_np)
    out = sharded(*dev_in, *make_dz())  # warmup
    jax.block_until_ready(out)
    times = []
    for _ in range(reps):
        dz = make_dz()
        t0 = time.perf_counter()
        out = sharded(*dev_in, *dz)
        jax.block_until_ready(out)
        times.append(time.perf_counter() - t0)
    return times


def time_device_pipelined(inputs_np, depth=64, rounds=2):
    """Per-exec wall time with `depth` executes in flight: amortizes the
    axon round-trip latency, giving the closest wall-clock estimate of
    per-execution device cost available on this tunneled setup."""
    import time

    sharded, dev_in, make_dz, jax = _timing_setup(inputs_np)
    out = sharded(*dev_in, *make_dz())  # warmup
    jax.block_until_ready(out)
    per_exec = []
    for _ in range(rounds):
        dzs = [make_dz() for _ in range(depth)]
        t0 = time.perf_counter()
        outs = [sharded(*dev_in, *dzs[i]) for i in range(depth)]
        jax.block_until_ready(outs)
        per_exec.append((time.perf_counter() - t0) / depth)
    return per_exec


def kernel(hidden_states, attention_mask, memory_slots, Wq, Wk, Wv, Wo):
    global LAST_RESULT
    hs = np.asarray(hidden_states, dtype=np.float32)
    mask = np.asarray(attention_mask)
    ms = np.asarray(memory_slots, dtype=np.float32)
    Wq = np.asarray(Wq, dtype=np.float32)
    Wk = np.asarray(Wk, dtype=np.float32)
    Wv = np.asarray(Wv, dtype=np.float32)
    Wo = np.asarray(Wo, dtype=np.float32)

    in_maps, cap = _prep_in_maps(hs, mask, ms, Wq, Wk, Wv, Wo)
    nc = _get_module(cap)

    kwargs = {}
    if TRACE:
        kwargs = {"trace": True}
        if TRACE_CORES is not None:
            kwargs["trace_cores"] = TRACE_CORES
    res = run_bass_kernel_spmd(nc, in_maps, core_ids=list(range(N_CORES)), **kwargs)
    LAST_RESULT = res

    y = _host_finish(res.results, Wv, Wo)
    return np.ascontiguousarray(y.astype(np.float32))
